# revision 4
# baseline (speedup 1.0000x reference)
"""Trainium2 Bass kernel for nn_ContrastiveLoss_76501957477132.

Math (see reference): with T=0.3, n=512 tracks, Q=8, M=8192, D=128,
  yf = y.reshape(nQ, D), y_idxs[k] = k % n, track_idxs[a] = a % n.
Per track i:
  num_xy[i] = sum_{a=i mod n} sum_{k=i mod n} exp(x_a.yf_k/T)
  den_xy[i] = sum_{a=i mod n} sum_k exp(x_a.yf_k/T) - num_xy[i]
  G[i]      = sum_{a=i mod n} sum_{m=i mod n} exp(x_a.x_m/T)
  num_xx[i] = (G[i] - diag_self[i]) / 2
  den_xx[i] = sum_{a=i mod n} sum_m exp(x_a.x_m/T) - G[i]
  loss = mean(-log(num/(num+den))) / Q

Because track/y labels are both (index mod 512) and 8192 = 16*512,
every "same-track" selection over a 512-aligned block is a block
diagonal.  The positive-pair terms (num_xy, G, diag_self) only touch
O(M*D) dot products -> computed exactly on the host in float64.

The device does the heavy part: the full row sums of exp(x@x.T/T) and
exp(x@yf.T/T).  Each core takes 1024 rows of x (8 subtiles of 128):
  matmul (bf16) -> PSUM fp32 [128, 2048] -> ScalarE exp(scale=1/T)
  with accum_out fusing the row reduction.  48 ACT instructions/core.
Host folds the per-(core, subtile) row sums by row residue (mod 512)
-- the "all-reduce" -- and finishes with the tiny log/mean.
"""

import numpy as np
import ml_dtypes

M, D, N_TRACKS, Q = 8192, 128, 512, 8
NQ = N_TRACKS * Q  # 4096
TEMP = 0.3
N_CORES = 8
ROWS_PER_CORE = M // N_CORES      # 1024
SUBTILES = ROWS_PER_CORE // 128   # 8
COLS = M + NQ                     # 12288 (xx cols then xy cols)
CHUNK = 2048                      # psum chunk (4 banks)
CHUNKS = COLS // CHUNK            # 6
ACTS_PER_CORE = SUBTILES * CHUNKS  # 48

_CACHED = {}


def _build_module():
    import concourse.bacc as bacc
    import concourse.tile as tile
    import concourse.mybir as mybir

    nc = bacc.Bacc(None, target_bir_lowering=False)
    bf16 = mybir.dt.bfloat16
    f32 = mybir.dt.float32

    lhsT_d = nc.dram_tensor("lhsT", [128, ROWS_PER_CORE], bf16, kind="ExternalInput")
    rhs_d = nc.dram_tensor("rhs", [128, COLS], bf16, kind="ExternalInput")
    acc_d = nc.dram_tensor("acc", [128, ACTS_PER_CORE], f32, kind="ExternalOutput")

    with tile.TileContext(nc) as tc:
        with (
            tc.tile_pool(name="consts", bufs=1) as consts,
            tc.tile_pool(name="accp", bufs=1) as accp,
            tc.tile_pool(name="psum", bufs=2, space="PSUM") as psum_pool,
        ):
            rhs_sb = consts.tile([128, COLS], bf16)
            nc.sync.dma_start(rhs_sb[:], rhs_d[:])
            lhsT_sb = consts.tile([128, ROWS_PER_CORE], bf16)
            nc.sync.dma_start(lhsT_sb[:], lhsT_d[:])
            acc_sb = accp.tile([128, ACTS_PER_CORE], f32)

            for sub in range(SUBTILES):
                lhsT_sub = lhsT_sb[:, sub * 128 : (sub + 1) * 128]
                for chunk in range(CHUNKS):
                    ps = psum_pool.tile([128, CHUNK], f32)
                    for j in range(CHUNK // 512):
                        col0 = chunk * CHUNK + j * 512
                        nc.tensor.matmul(
                            ps[:, j * 512 : (j + 1) * 512],
                            lhsT_sub,
                            rhs_sb[:, col0 : col0 + 512],
                            start=True,
                            stop=True,
                        )
                    slot = sub * CHUNKS + chunk
                    nc.scalar.activation(
                        out=ps[:],
                        in_=ps[:],
                        func=mybir.ActivationFunctionType.Exp,
                        scale=1.0 / TEMP,
                        accum_out=acc_sb[:, slot : slot + 1],
                    )
            nc.sync.dma_start(acc_d[:], acc_sb[:])
    nc.compile()
    return nc


def _get_module():
    if "nc" not in _CACHED:
        _CACHED["nc"] = _build_module()
    return _CACHED["nc"]


def _positive_terms(x64, yf64):
    """num_xy, G_diag, diag_self as float64 [512] vectors (exact math)."""
    xs = x64.reshape(M // N_TRACKS, N_TRACKS, D)        # [16, 512, 128]
    yfs = yf64.reshape(NQ // N_TRACKS, N_TRACKS, D)     # [8, 512, 128]
    dxx = np.einsum("rid,cid->rci", xs, xs)             # [16, 16, 512]
    dxy = np.einsum("rid,qid->rqi", xs, yfs)            # [16, 8, 512]
    G = np.exp(dxx / TEMP).sum(axis=(0, 1))             # [512]
    diag_self = np.exp(np.einsum("rid,rid->ri", xs, xs) / TEMP).sum(axis=0)
    num_xy = np.exp(dxy / TEMP).sum(axis=(0, 1))        # [512]
    return num_xy, G, diag_self


def _finish(rs_seg, num_xy, G, diag_self):
    num = num_xy + (G - diag_self) / 2.0
    den = rs_seg - num_xy - G
    loss = np.mean(-np.log(num / (den + num))) / Q
    return np.asarray(loss, dtype=np.float32)


def _numpy_fallback(x, track_idxs, y):
    """Exact general-track reference in numpy (safety net only)."""
    x64 = x.astype(np.float64)
    yf64 = y.reshape(NQ, D).astype(np.float64)
    t = track_idxs.astype(np.int64)
    y_idxs = np.tile(np.arange(N_TRACKS, dtype=np.int64), Q)
    E_xy = np.exp(x64 @ yf64.T / TEMP)
    Sx = np.zeros((N_TRACKS, NQ))
    np.add.at(Sx, t, E_xy)
    Py = (y_idxs[:, None] == np.arange(N_TRACKS)[None, :]).astype(np.float64)
    num_xy = np.einsum("ik,ki->i", Sx, Py)
    den_xy = Sx.sum(axis=1) - num_xy
    E_xx = np.exp(x64 @ x64.T / TEMP)
    Sxx = np.zeros((N_TRACKS, M))
    np.add.at(Sxx, t, E_xx)
    Px = (t[:, None] == np.arange(N_TRACKS)[None, :]).astype(np.float64)
    G_diag = np.einsum("im,mi->i", Sxx, Px)
    diag_self = np.zeros(N_TRACKS)
    np.add.at(diag_self, t, np.diagonal(E_xx))
    num_xx = (G_diag - diag_self) / 2.0
    den_xx = Sxx.sum(axis=1) - G_diag
    num = num_xy + num_xx
    den = den_xy + den_xx
    loss = np.mean(-np.log(num / (den + num))) / Q
    return np.asarray(loss, dtype=np.float32)


def kernel(x, track_idxs, y):
    x = np.asarray(x, dtype=np.float32)
    y = np.asarray(y, dtype=np.float32)
    track_idxs = np.asarray(track_idxs)

    expected_tracks = (np.arange(M, dtype=np.int64) % N_TRACKS)
    if (
        x.shape != (M, D)
        or y.shape != (N_TRACKS, Q, D)
        or not np.array_equal(track_idxs.astype(np.int64), expected_tracks)
    ):
        return _numpy_fallback(x, track_idxs, y)

    from concourse.bass_utils import run_bass_kernel_spmd

    yf = np.ascontiguousarray(y.reshape(NQ, D))
    xT = np.ascontiguousarray(x.T)       # [128, 8192]
    yfT = np.ascontiguousarray(yf.T)     # [128, 4096]
    rhs = np.concatenate([xT, yfT], axis=1).astype(ml_dtypes.bfloat16)  # [128, 12288]

    in_maps = []
    for k in range(N_CORES):
        lhsT = np.ascontiguousarray(rhs[:, k * ROWS_PER_CORE : (k + 1) * ROWS_PER_CORE])
        in_maps.append({"lhsT": lhsT, "rhs": rhs})

    nc = _get_module()
    res = run_bass_kernel_spmd(nc, in_maps, core_ids=list(range(N_CORES)))
    _CACHED["last_res"] = res

    # Fold per-(core, subtile, chunk) row sums by row residue (mod 512).
    # Row (k*1024 + sub*128 + p) has residue 128*(sub%4) + p.
    rs_seg = np.zeros(N_TRACKS, dtype=np.float64)
    for k in range(N_CORES):
        acc = np.asarray(res.results[k]["acc"], dtype=np.float64)  # [128, 48]
        per_sub = acc.reshape(128, SUBTILES, CHUNKS).sum(axis=2)   # [128 p, 8 sub]
        folded = per_sub.reshape(128, 2, 4).sum(axis=1)            # [128 p, 4 t4]
        rs_seg += folded.T.reshape(N_TRACKS)                       # i = 128*t4 + p

    num_xy, G, diag_self = _positive_terms(
        x.astype(np.float64), yf.astype(np.float64)
    )
    return _finish(rs_seg, num_xy, G, diag_self)


# revision 6
# speedup vs baseline: 1.1956x; 1.1956x over previous
"""Trainium2 Bass kernel for nn_ContrastiveLoss_76501957477132.

Math (see reference): with T=0.3, n=512 tracks, Q=8, M=8192, D=128,
  yf = y.reshape(nQ, D), y_idxs[k] = k % n, track_idxs[a] = a % n.
Per track i:
  num_xy[i] = sum_{a=i mod n} sum_{k=i mod n} exp(x_a.yf_k/T)
  den_xy[i] = sum_{a=i mod n} sum_k exp(x_a.yf_k/T) - num_xy[i]
  G[i]      = sum_{a=i mod n} sum_{m=i mod n} exp(x_a.x_m/T)
  num_xx[i] = (G[i] - diag_self[i]) / 2
  den_xx[i] = sum_{a=i mod n} sum_m exp(x_a.x_m/T) - G[i]
  loss = mean(-log(num/(num+den))) / Q

Track labels are (row index mod 512) and 8192 = 16*512, so all
"positive pair" selections over 512-aligned blocks are block
diagonals.  Those terms only touch O(M*D) dot products and are
computed exactly on the host in float64.

The device computes the heavy denominators: per-track sums of
exp(x@x.T/T) and exp(x@yf.T/T).  E_xx is symmetric, so only the
upper-triangle 512x512 blocks are computed: a block's ACT accum_out
rowsum covers its own rows, and a ones-vector matmul (colsum on the
tensor engine) covers the mirrored rows.  Work is cut into
[128 x 512] "units" (matmul lhsT = 128 x-rows, rhs = one 512-row
block of xT/yfT).  Unit count per core is exactly 132 = 4 residue
groups x (15 off-diag xx + 2 diag xx + 16 xy) via band pairing
(k, 15-k), so one SPMD program serves all 8 cores; per-unit
lhsT/rhs are host-gathered inputs.

Pipeline per chunk of 3 units: 3 matmuls (bf16) -> PSUM fp32
[128,1536] -> ScalarE exp(scale=1/T, accum_out=rowsums) -> bf16
scratch; off-diag chunks additionally run 3 ones-matmuls on the
scratch accumulating colsums into a persistent [1,512] PSUM bank.
Host folds rowsum/colsum partials by row residue (mod 512) -- the
"all-reduce" -- and finishes with the tiny log/mean.
"""

import numpy as np
import ml_dtypes

M, D, N_TRACKS, Q = 8192, 128, 512, 8
NQ = N_TRACKS * Q  # 4096
TEMP = 0.3
N_CORES = 8
N_BANDS = M // N_TRACKS           # 16 row/col bands of 512
GROUPS = 4                        # residue groups (s): rows 128s..128s+127 of a band
UNITS = 33                        # units per group: 15 off + 2 diag + 16 xy
OFF_UNITS = 15
CHUNK_UNITS = 3                   # units per psum chunk [128, 1536]
CHUNKS_PER_GROUP = UNITS // CHUNK_UNITS  # 11
OFF_CHUNKS = OFF_UNITS // CHUNK_UNITS    # 5
ACTS_PER_CORE = GROUPS * CHUNKS_PER_GROUP  # 44

_CACHED = {}


def _core_units(k):
    """Unit descriptors for core k: list of (band, rhs_kind, rhs_idx).

    rhs_kind: 'x' -> xT block rhs_idx, 'y' -> yfT block rhs_idx.
    Order: 15 off-diag xx, 2 diag xx, 16 xy.  Bands A=k, B=15-k.
    """
    A, B = k, (N_BANDS - 1) - k
    units = []
    units += [(A, "x", c) for c in range(A + 1, N_BANDS)]   # 15-k
    units += [(B, "x", c) for c in range(B + 1, N_BANDS)]   # k
    assert len(units) == OFF_UNITS
    units += [(A, "x", A), (B, "x", B)]                     # diag blocks
    units += [(A, "y", q) for q in range(Q)]
    units += [(B, "y", q) for q in range(Q)]
    assert len(units) == UNITS
    return units


def _build_module():
    import concourse.bacc as bacc
    import concourse.tile as tile
    import concourse.mybir as mybir

    nc = bacc.Bacc(None, target_bir_lowering=False)
    bf16 = mybir.dt.bfloat16
    f32 = mybir.dt.float32

    # per-unit stationary operands, one [128,128] slice per (group, unit)
    lhsT_d = nc.dram_tensor(
        "lhsT", [128, GROUPS, UNITS, 128], bf16, kind="ExternalInput"
    )
    rhs_d = nc.dram_tensor("rhs", [128, UNITS, 512], bf16, kind="ExternalInput")
    acc_d = nc.dram_tensor("acc", [128, ACTS_PER_CORE], f32, kind="ExternalOutput")
    cs_d = nc.dram_tensor("cs", [1, 512], f32, kind="ExternalOutput")

    with tile.TileContext(nc) as tc:
        with (
            tc.tile_pool(name="consts", bufs=1) as consts,
            tc.tile_pool(name="accp", bufs=1) as accp,
            tc.tile_pool(name="scratch", bufs=3) as scratch_pool,
            tc.tile_pool(name="psum", bufs=2, space="PSUM") as psum_pool,
            tc.tile_pool(name="cspsum", bufs=1, space="PSUM") as cs_pool,
        ):
            # split input DMAs so compute starts early
            rhs_sbs, rhs_map = [], {}
            splits = [(0, 6), (6, 20), (20, UNITS)]
            for lo, hi in splits:
                t = consts.tile([128, hi - lo, 512], bf16, tag=f"rhs{lo}")
                nc.sync.dma_start(t[:], rhs_d[:, lo:hi, :])
                rhs_sbs.append(t)
                for u in range(lo, hi):
                    rhs_map[u] = (t, u - lo)

            lhsT_sbs = []
            for s in range(GROUPS):
                t = consts.tile([128, UNITS, 128], bf16, tag=f"lhsT{s}")
                nc.sync.dma_start(t[:], lhsT_d[:, s, :, :])
                lhsT_sbs.append(t)

            ones_sb = consts.tile([128, 1], bf16, tag="ones")
            nc.vector.memset(ones_sb[:], 1.0)

            acc_sb = accp.tile([128, ACTS_PER_CORE], f32)
            cs_ps = cs_pool.tile([1, 512], f32)

            n_ones = GROUPS * OFF_CHUNKS * CHUNK_UNITS  # 60
            ones_done = 0
            pending = []  # delayed ones-matmuls: (scratch_tile, slice_idx)

            def flush_pending():
                nonlocal ones_done
                for sc, j in pending:
                    ones_done += 1
                    nc.tensor.matmul(
                        cs_ps[:],
                        ones_sb[:],
                        sc[:, j * 512 : (j + 1) * 512],
                        start=(ones_done == 1),
                        stop=(ones_done == n_ones),
                        skip_group_check=True,
                    )
                pending.clear()

            for s in range(GROUPS):
                for j in range(CHUNKS_PER_GROUP):
                    ps = psum_pool.tile([128, CHUNK_UNITS * 512], f32)
                    for e in range(CHUNK_UNITS):
                        u = j * CHUNK_UNITS + e
                        rt, ri = rhs_map[u]
                        nc.tensor.matmul(
                            ps[:, e * 512 : (e + 1) * 512],
                            lhsT_sbs[s][:, u, :],
                            rt[:, ri, :],
                            start=True,
                            stop=True,
                        )
                    # ones-matmuls for the previous off-chunk run after this
                    # chunk's matmuls so the PE never waits on the ACT
                    flush_pending()
                    slot = s * CHUNKS_PER_GROUP + j
                    sc = scratch_pool.tile([128, CHUNK_UNITS * 512], bf16)
                    nc.scalar.activation(
                        out=sc[:],
                        in_=ps[:],
                        func=mybir.ActivationFunctionType.Exp,
                        scale=1.0 / TEMP,
                        accum_out=acc_sb[:, slot : slot + 1],
                    )
                    if j < OFF_CHUNKS:
                        pending.extend((sc, e) for e in range(CHUNK_UNITS))
            flush_pending()

            nc.sync.dma_start(acc_d[:], acc_sb[:])
            cs_sb = accp.tile([1, 512], f32, tag="cs_sb")
            nc.vector.tensor_copy(cs_sb[:], cs_ps[:])
            nc.sync.dma_start(cs_d[:], cs_sb[:])
    nc.compile()
    return nc


def _get_module():
    if "nc" not in _CACHED:
        _CACHED["nc"] = _build_module()
    return _CACHED["nc"]


def _positive_terms(x64, yf64):
    """num_xy, G_diag, diag_self as float64 [512] vectors (exact math)."""
    xs = x64.reshape(N_BANDS, N_TRACKS, D)              # [16, 512, 128]
    yfs = yf64.reshape(NQ // N_TRACKS, N_TRACKS, D)     # [8, 512, 128]
    dxx = np.einsum("rid,cid->rci", xs, xs)             # [16, 16, 512]
    dxy = np.einsum("rid,qid->rqi", xs, yfs)            # [16, 8, 512]
    G = np.exp(dxx / TEMP).sum(axis=(0, 1))             # [512]
    diag_self = np.exp(np.einsum("rid,rid->ri", xs, xs) / TEMP).sum(axis=0)
    num_xy = np.exp(dxy / TEMP).sum(axis=(0, 1))        # [512]
    return num_xy, G, diag_self


def _finish(rs_seg, num_xy, G, diag_self):
    num = num_xy + (G - diag_self) / 2.0
    den = rs_seg - num_xy - G
    loss = np.mean(-np.log(num / (den + num))) / Q
    return np.asarray(loss, dtype=np.float32)


def _numpy_fallback(x, track_idxs, y):
    """Exact general-track reference in numpy (safety net only)."""
    x64 = x.astype(np.float64)
    yf64 = y.reshape(NQ, D).astype(np.float64)
    t = track_idxs.astype(np.int64)
    y_idxs = np.tile(np.arange(N_TRACKS, dtype=np.int64), Q)
    E_xy = np.exp(x64 @ yf64.T / TEMP)
    Sx = np.zeros((N_TRACKS, NQ))
    np.add.at(Sx, t, E_xy)
    Py = (y_idxs[:, None] == np.arange(N_TRACKS)[None, :]).astype(np.float64)
    num_xy = np.einsum("ik,ki->i", Sx, Py)
    den_xy = Sx.sum(axis=1) - num_xy
    E_xx = np.exp(x64 @ x64.T / TEMP)
    Sxx = np.zeros((N_TRACKS, M))
    np.add.at(Sxx, t, E_xx)
    Px = (t[:, None] == np.arange(N_TRACKS)[None, :]).astype(np.float64)
    G_diag = np.einsum("im,mi->i", Sxx, Px)
    diag_self = np.zeros(N_TRACKS)
    np.add.at(diag_self, t, np.diagonal(E_xx))
    num_xx = (G_diag - diag_self) / 2.0
    den_xx = Sxx.sum(axis=1) - G_diag
    num = num_xy + num_xx
    den = den_xy + den_xx
    loss = np.mean(-np.log(num / (den + num))) / Q
    return np.asarray(loss, dtype=np.float32)


def kernel(x, track_idxs, y):
    x = np.asarray(x, dtype=np.float32)
    y = np.asarray(y, dtype=np.float32)
    track_idxs = np.asarray(track_idxs)

    expected_tracks = np.arange(M, dtype=np.int64) % N_TRACKS
    if (
        x.shape != (M, D)
        or y.shape != (N_TRACKS, Q, D)
        or not np.array_equal(track_idxs.astype(np.int64), expected_tracks)
    ):
        return _numpy_fallback(x, track_idxs, y)

    from concourse.bass_utils import run_bass_kernel_spmd

    yf = np.ascontiguousarray(y.reshape(NQ, D))
    xT = np.ascontiguousarray(x.T).astype(ml_dtypes.bfloat16)    # [128, 8192]
    yfT = np.ascontiguousarray(yf.T).astype(ml_dtypes.bfloat16)  # [128, 4096]
    xT_blocks = xT.reshape(128, N_BANDS, 512)
    yfT_blocks = yfT.reshape(128, Q, 512)

    in_maps = []
    for k in range(N_CORES):
        units = _core_units(k)
        rhs = np.stack(
            [
                (xT_blocks[:, idx] if kind == "x" else yfT_blocks[:, idx])
                for (_band, kind, idx) in units
            ],
            axis=1,
        )  # [128, 33, 512]
        lhsT = np.empty((128, GROUPS, UNITS, 128), dtype=ml_dtypes.bfloat16)
        for s in range(GROUPS):
            for u, (band, _kind, _idx) in enumerate(units):
                t = 4 * band + s  # global row-subtile
                lhsT[:, s, u, :] = xT[:, 128 * t : 128 * (t + 1)]
        in_maps.append(
            {"lhsT": np.ascontiguousarray(lhsT), "rhs": np.ascontiguousarray(rhs)}
        )

    nc = _get_module()
    res = run_bass_kernel_spmd(nc, in_maps, core_ids=list(range(N_CORES)))
    _CACHED["last_res"] = res

    # Fold partial sums by row residue (mod 512): group s covers residues
    # 128s + p; colsums fold by in-block column position directly.
    rs_seg = np.zeros(N_TRACKS, dtype=np.float64)
    for k in range(N_CORES):
        acc = np.asarray(res.results[k]["acc"], dtype=np.float64)  # [128, 44]
        per_group = acc.reshape(128, GROUPS, CHUNKS_PER_GROUP).sum(axis=2)
        rs_seg += per_group.T.reshape(N_TRACKS)  # i = 128*s + p
        rs_seg += np.asarray(res.results[k]["cs"], dtype=np.float64).reshape(-1)

    num_xy, G, diag_self = _positive_terms(
        x.astype(np.float64), yf.astype(np.float64)
    )
    return _finish(rs_seg, num_xy, G, diag_self)


# revision 11
# speedup vs baseline: 1.2959x; 1.0839x over previous
"""Trainium2 Bass kernel for nn_ContrastiveLoss_76501957477132.

Math (see reference): with T=0.3, n=512 tracks, Q=8, M=8192, D=128,
  yf = y.reshape(nQ, D), y_idxs[k] = k % n, track_idxs[a] = a % n.
Per track i:
  num_xy[i] = sum_{a=i mod n} sum_{k=i mod n} exp(x_a.yf_k/T)
  den_xy[i] = sum_{a=i mod n} sum_k exp(x_a.yf_k/T) - num_xy[i]
  G[i]      = sum_{a=i mod n} sum_{m=i mod n} exp(x_a.x_m/T)
  num_xx[i] = (G[i] - diag_self[i]) / 2
  den_xx[i] = sum_{a=i mod n} sum_m exp(x_a.x_m/T) - G[i]
  loss = mean(-log(num/(num+den))) / Q

Track labels are (row index mod 512) and 8192 = 16*512, so all
"positive pair" selections over 512-aligned blocks are block
diagonals.  Those terms only touch O(M*D) dot products and are
computed exactly on the host in float64.

The device computes the heavy denominators: per-track sums of
exp(x@x.T/T) and exp(x@yf.T/T).  E_xx is symmetric, so only the
upper-triangle 512x512 blocks are computed: a block's ACT accum_out
rowsum covers its own rows, and a ones-vector matmul (colsum on the
tensor engine) covers the mirrored rows.  Work is cut into
[128 x 512] "units" (matmul lhsT = 128 x-rows, rhs = one 512-row
block of xT/yfT).  Unit count per core is exactly 132 = 4 residue
groups x (15 off-diag xx + 2 diag xx + 16 xy) via band pairing
(k, 15-k), so one SPMD program serves all 8 cores; per-unit
lhsT/rhs are host-gathered inputs.

Pipeline per chunk of 3 units: 3 matmuls (bf16) -> PSUM fp32
[128,1536] -> ScalarE exp(scale=1/T, accum_out=rowsums) -> bf16
scratch; off-diag chunks additionally run 3 ones-matmuls on the
scratch accumulating colsums into a persistent [1,512] PSUM bank.
Host folds rowsum/colsum partials by row residue (mod 512) -- the
"all-reduce" -- and finishes with the tiny log/mean.
"""

import numpy as np
import ml_dtypes

M, D, N_TRACKS, Q = 8192, 128, 512, 8
NQ = N_TRACKS * Q  # 4096
TEMP = 0.3
N_CORES = 8
N_BANDS = M // N_TRACKS           # 16 row/col bands of 512
GROUPS = 4                        # residue groups (s): rows 128s..128s+127 of a band
UNITS = 33                        # units per group: 15 off + 2 diag + 16 xy
OFF_UNITS = 15
CHUNK_UNITS = 3                   # units per psum chunk [128, 1536]
CHUNKS_PER_GROUP = UNITS // CHUNK_UNITS  # 11
OFF_CHUNKS = OFF_UNITS // CHUNK_UNITS    # 5
ACTS_PER_CORE = GROUPS * CHUNKS_PER_GROUP  # 44

_CACHED = {}


def _core_units(k):
    """Unit descriptors for core k: list of (band, rhs_kind, rhs_idx).

    rhs_kind: 'x' -> xT block rhs_idx, 'y' -> yfT block rhs_idx.
    Order: 15 off-diag xx, 2 diag xx, 16 xy.  Bands A=k, B=15-k.
    """
    A, B = k, (N_BANDS - 1) - k
    units = []
    units += [(A, "x", c) for c in range(A + 1, N_BANDS)]   # 15-k
    units += [(B, "x", c) for c in range(B + 1, N_BANDS)]   # k
    assert len(units) == OFF_UNITS
    units += [(A, "x", A), (B, "x", B)]                     # diag blocks
    units += [(A, "y", q) for q in range(Q)]
    units += [(B, "y", q) for q in range(Q)]
    assert len(units) == UNITS
    return units


def _build_module():
    import concourse.bacc as bacc
    import concourse.tile as tile
    import concourse.mybir as mybir

    nc = bacc.Bacc(None, target_bir_lowering=False)
    bf16 = mybir.dt.bfloat16
    f32 = mybir.dt.float32

    # per-unit stationary operands, one [128,128] slice per (group, unit).
    # rhs blocks are deduplicated: A-xy and B-xy units share yfT blocks, so
    # only 25 distinct 512-column blocks are stored (17 xT + 8 yfT).
    RHS_BLOCKS = UNITS - Q  # 25
    lhsT_d = nc.dram_tensor(
        "lhsT", [128, GROUPS, UNITS, 128], bf16, kind="ExternalInput"
    )
    rhs_d = nc.dram_tensor("rhs", [128, RHS_BLOCKS, 512], bf16, kind="ExternalInput")
    acc_d = nc.dram_tensor("acc", [128, ACTS_PER_CORE], f32, kind="ExternalOutput")
    cs_d = nc.dram_tensor("cs", [1, 512], f32, kind="ExternalOutput")

    with tile.TileContext(nc) as tc:
        with (
            tc.tile_pool(name="consts", bufs=1) as consts,
            tc.tile_pool(name="accp", bufs=1) as accp,
            tc.tile_pool(name="scratch", bufs=3) as scratch_pool,
            tc.tile_pool(name="psum", bufs=2, space="PSUM") as psum_pool,
            tc.tile_pool(name="cspsum", bufs=1, space="PSUM") as cs_pool,
        ):
            # Input DMAs split into consumption-ordered pieces spread over
            # two DGE queues (SP hardware DGE + GpSimd software DGE; Scalar
            # stays free for the exp stream) so the first matmul starts
            # within ~2us and the rest of the load hides under compute.
            dma_engines = [nc.sync, nc.gpsimd]
            dma_i = 0

            def dma(out_ap, in_ap):
                nonlocal dma_i
                dma_engines[dma_i % len(dma_engines)].dma_start(out_ap, in_ap)
                dma_i += 1

            rhs_splits = [(0, 3), (3, 9), (9, 15), (15, 21), (21, RHS_BLOCKS)]
            rhs_tiles = {}
            for lo, hi in rhs_splits:
                rhs_tiles[lo] = consts.tile(
                    [128, hi - lo, 512], bf16, tag=f"rhs{lo}", name=f"rhs{lo}"
                )

            lhsT_splits = [(0, 9), (9, UNITS)]
            lhsT_tiles = {}
            for s in range(GROUPS):
                for lo, hi in lhsT_splits:
                    lhsT_tiles[(s, lo)] = consts.tile(
                        [128, hi - lo, 128],
                        bf16,
                        tag=f"lhsT{s}_{lo}",
                        name=f"lhsT{s}_{lo}",
                    )

            # consumption order: group 0 needs rhs pieces + its lhsT first
            dma(rhs_tiles[0][:], rhs_d[:, 0:3, :])
            dma(lhsT_tiles[(0, 0)][:], lhsT_d[:, 0, 0:9, :])
            dma(rhs_tiles[3][:], rhs_d[:, 3:9, :])
            dma(lhsT_tiles[(0, 9)][:], lhsT_d[:, 0, 9:UNITS, :])
            dma(rhs_tiles[9][:], rhs_d[:, 9:15, :])
            dma(rhs_tiles[15][:], rhs_d[:, 15:21, :])
            dma(rhs_tiles[21][:], rhs_d[:, 21:RHS_BLOCKS, :])
            for s in range(1, GROUPS):
                for lo, hi in lhsT_splits:
                    dma(lhsT_tiles[(s, lo)][:], lhsT_d[:, s, lo:hi, :])

            def rhs_ap(u):
                blk = u if u < 25 else u - Q  # B-xy units reuse yfT blocks
                for lo, hi in rhs_splits:
                    if lo <= blk < hi:
                        return rhs_tiles[lo][:, blk - lo, :]
                raise AssertionError

            def lhsT_ap(s, u):
                for lo, hi in lhsT_splits:
                    if lo <= u < hi:
                        return lhsT_tiles[(s, lo)][:, u - lo, :]
                raise AssertionError

            ones_sb = consts.tile([128, 1], bf16, tag="ones")
            nc.vector.memset(ones_sb[:], 1.0)

            acc_sb = accp.tile([128, ACTS_PER_CORE], f32)
            cs_ps = cs_pool.tile([1, 512], f32)

            n_ones = GROUPS * OFF_CHUNKS * CHUNK_UNITS  # 60
            ones_done = 0
            pending = []  # delayed ones-matmuls: (scratch_tile, slice_idx)

            def flush_pending():
                nonlocal ones_done
                for sc, j in pending:
                    ones_done += 1
                    nc.tensor.matmul(
                        cs_ps[:],
                        ones_sb[:],
                        sc[:, j * 512 : (j + 1) * 512],
                        start=(ones_done == 1),
                        stop=(ones_done == n_ones),
                        skip_group_check=True,
                    )
                pending.clear()

            for s in range(GROUPS):
                for j in range(CHUNKS_PER_GROUP):
                    ps = psum_pool.tile([128, CHUNK_UNITS * 512], f32)
                    for e in range(CHUNK_UNITS):
                        u = j * CHUNK_UNITS + e
                        nc.tensor.matmul(
                            ps[:, e * 512 : (e + 1) * 512],
                            lhsT_ap(s, u),
                            rhs_ap(u),
                            start=True,
                            stop=True,
                        )
                    # ones-matmuls for the previous off-chunk run after this
                    # chunk's matmuls so the PE never waits on the ACT
                    flush_pending()
                    slot = s * CHUNKS_PER_GROUP + j
                    sc = scratch_pool.tile([128, CHUNK_UNITS * 512], bf16)
                    nc.scalar.activation(
                        out=sc[:],
                        in_=ps[:],
                        func=mybir.ActivationFunctionType.Exp,
                        scale=1.0 / TEMP,
                        accum_out=acc_sb[:, slot : slot + 1],
                    )
                    if j < OFF_CHUNKS:
                        pending.extend((sc, e) for e in range(CHUNK_UNITS))
            flush_pending()

            nc.sync.dma_start(acc_d[:], acc_sb[:])
            cs_sb = accp.tile([1, 512], f32, tag="cs_sb")
            nc.vector.tensor_copy(cs_sb[:], cs_ps[:])
            nc.sync.dma_start(cs_d[:], cs_sb[:])
    nc.compile()
    return nc


def _get_module():
    if "nc" not in _CACHED:
        _CACHED["nc"] = _build_module()
    return _CACHED["nc"]


def _positive_terms(x64, yf64):
    """num_xy, G_diag, diag_self as float64 [512] vectors (exact math)."""
    xs = x64.reshape(N_BANDS, N_TRACKS, D)              # [16, 512, 128]
    yfs = yf64.reshape(NQ // N_TRACKS, N_TRACKS, D)     # [8, 512, 128]
    dxx = np.einsum("rid,cid->rci", xs, xs)             # [16, 16, 512]
    dxy = np.einsum("rid,qid->rqi", xs, yfs)            # [16, 8, 512]
    G = np.exp(dxx / TEMP).sum(axis=(0, 1))             # [512]
    diag_self = np.exp(np.einsum("rid,rid->ri", xs, xs) / TEMP).sum(axis=0)
    num_xy = np.exp(dxy / TEMP).sum(axis=(0, 1))        # [512]
    return num_xy, G, diag_self


def _finish(rs_seg, num_xy, G, diag_self):
    num = num_xy + (G - diag_self) / 2.0
    den = rs_seg - num_xy - G
    loss = np.mean(-np.log(num / (den + num))) / Q
    return np.asarray(loss, dtype=np.float32)


def _numpy_fallback(x, track_idxs, y):
    """Exact general-track reference in numpy (safety net only)."""
    x64 = x.astype(np.float64)
    yf64 = y.reshape(NQ, D).astype(np.float64)
    t = track_idxs.astype(np.int64)
    y_idxs = np.tile(np.arange(N_TRACKS, dtype=np.int64), Q)
    E_xy = np.exp(x64 @ yf64.T / TEMP)
    Sx = np.zeros((N_TRACKS, NQ))
    np.add.at(Sx, t, E_xy)
    Py = (y_idxs[:, None] == np.arange(N_TRACKS)[None, :]).astype(np.float64)
    num_xy = np.einsum("ik,ki->i", Sx, Py)
    den_xy = Sx.sum(axis=1) - num_xy
    E_xx = np.exp(x64 @ x64.T / TEMP)
    Sxx = np.zeros((N_TRACKS, M))
    np.add.at(Sxx, t, E_xx)
    Px = (t[:, None] == np.arange(N_TRACKS)[None, :]).astype(np.float64)
    G_diag = np.einsum("im,mi->i", Sxx, Px)
    diag_self = np.zeros(N_TRACKS)
    np.add.at(diag_self, t, np.diagonal(E_xx))
    num_xx = (G_diag - diag_self) / 2.0
    den_xx = Sxx.sum(axis=1) - G_diag
    num = num_xy + num_xx
    den = den_xy + den_xx
    loss = np.mean(-np.log(num / (den + num))) / Q
    return np.asarray(loss, dtype=np.float32)


def kernel(x, track_idxs, y):
    x = np.asarray(x, dtype=np.float32)
    y = np.asarray(y, dtype=np.float32)
    track_idxs = np.asarray(track_idxs)

    expected_tracks = np.arange(M, dtype=np.int64) % N_TRACKS
    if (
        x.shape != (M, D)
        or y.shape != (N_TRACKS, Q, D)
        or not np.array_equal(track_idxs.astype(np.int64), expected_tracks)
    ):
        return _numpy_fallback(x, track_idxs, y)

    from concourse.bass_utils import run_bass_kernel_spmd

    yf = np.ascontiguousarray(y.reshape(NQ, D))
    xT = np.ascontiguousarray(x.T).astype(ml_dtypes.bfloat16)    # [128, 8192]
    yfT = np.ascontiguousarray(yf.T).astype(ml_dtypes.bfloat16)  # [128, 4096]
    xT_blocks = xT.reshape(128, N_BANDS, 512)
    yfT_blocks = yfT.reshape(128, Q, 512)

    in_maps = []
    for k in range(N_CORES):
        units = _core_units(k)
        rhs = np.stack(
            [
                (xT_blocks[:, idx] if kind == "x" else yfT_blocks[:, idx])
                for (_band, kind, idx) in units[: UNITS - Q]
            ],
            axis=1,
        )  # [128, 25, 512]: 17 xT blocks + 8 yfT blocks (shared A/B xy)
        lhsT = np.empty((128, GROUPS, UNITS, 128), dtype=ml_dtypes.bfloat16)
        for s in range(GROUPS):
            for u, (band, _kind, _idx) in enumerate(units):
                t = 4 * band + s  # global row-subtile
                lhsT[:, s, u, :] = xT[:, 128 * t : 128 * (t + 1)]
        in_maps.append(
            {"lhsT": np.ascontiguousarray(lhsT), "rhs": np.ascontiguousarray(rhs)}
        )

    nc = _get_module()
    res = run_bass_kernel_spmd(nc, in_maps, core_ids=list(range(N_CORES)))
    _CACHED["last_res"] = res

    # Fold partial sums by row residue (mod 512): group s covers residues
    # 128s + p; colsums fold by in-block column position directly.
    rs_seg = np.zeros(N_TRACKS, dtype=np.float64)
    for k in range(N_CORES):
        acc = np.asarray(res.results[k]["acc"], dtype=np.float64)  # [128, 44]
        per_group = acc.reshape(128, GROUPS, CHUNKS_PER_GROUP).sum(axis=2)
        rs_seg += per_group.T.reshape(N_TRACKS)  # i = 128*s + p
        rs_seg += np.asarray(res.results[k]["cs"], dtype=np.float64).reshape(-1)

    num_xy, G, diag_self = _positive_terms(
        x.astype(np.float64), yf.astype(np.float64)
    )
    return _finish(rs_seg, num_xy, G, diag_self)
